# revision 26
# baseline (speedup 1.0000x reference)
"""Causal self-attention block (RMSNorm + QKV + RoPE + causal attention +
out-proj + residual) on 8 Trainium2 NeuronCores.

Sharding: batch (B=2) x head-groups (16 heads -> 4 groups of 4) = 8 shards.
Core c handles batch b = c // 4 and heads [4*(c%4), 4*(c%4)+4).
Each core computes RMSNorm(x_b), its 4 heads' Q/K/V projections, RoPE,
causal attention, and a partial out-projection over its 256-dim slice of
the concatenated head outputs.  The host sums the 4 partials per batch and
adds the residual (the all-reduce after out_proj, done during the gather).

Layout notes:
 - The host ships x twice: row-major (RMS stats via ACT square+accum) and
   pre-transposed x^T (d on partitions) which feeds the QKV matmuls as the
   stationary operand directly - no on-device x transposes.  Both x and
   the qkv weights travel as fp8e4 (weights pre-scaled by 64 to clear the
   subnormal range; the descale rides the 1/rms factor), so the QKV
   projection runs in DoubleRow mode - two d-blocks per PE pass.
 - All attention operands live transposed (head_dim on partitions): Q^T/K^T
   are built by PE transposes of the projection output; RoPE is applied in
   the transposed domain with per-partition cos/sin tables (host-replicated
   per head-block so the combine runs as a few wide DVE ops).
 - scores^T (k on partitions, q free) lets softmax skip max-subtraction
   (scores are O(3) here) and the ones-column appended to V yields the
   softmax denominators from the same PV matmul.  Score tiles for adjacent
   k-tiles share one 2-bank PSUM tile so each exp activation covers two
   k-tiles (halves the ACT fixed overhead), exp writes probabilities as
   fp8, and the PV matmul contracts both k-tiles of a pair in one
   DoubleRow pass.
 - Emission spreads projection (next chunk) and out-projection (previous
   chunk) tiles between the per-head attention groups so the PE always has
   independent matmul work while ACT chews through the exps; a short burst
   of dummy matmuls at kernel start warms the PE clock (HAM) while the
   first DMAs land.

Self-contained: hardcodes all shapes; no sibling imports.
"""

import numpy as np

import ml_dtypes

import concourse.bacc as bacc
import concourse.tile as tile
from concourse import mybir
from concourse.bass_utils import run_bass_kernel_spmd
from concourse.masks import make_identity

# Problem shapes (hardcoded per contract)
B, T, D, NHEADS = 2, 2048, 1024, 16
HEAD_DIM = 64
EPS = 1e-6
ROPE_BASE = 10000.0

HL = 4          # heads per core
E3 = 3 * HL * HEAD_DIM  # 768 local qkv output dims
P = 128
NT = T // P     # 16 t-tiles
ND = D // P     # 8 d-tiles of the model dim
NQC = T // 512  # 4 query chunks
NCORES = 8

F32 = mybir.dt.float32
BF16 = mybir.dt.bfloat16
FP8 = mybir.dt.float8e4

MM_DT = BF16    # scores / attention-out / out-proj operand dtype
X_DT = FP8      # x, x^T, w_qkv (DoubleRow projection)
TAB_DT = BF16   # rope tables
OUT_DT = BF16
WS = 64.0       # host-side w_qkv scale (fp8 subnormal avoidance)
VW = HEAD_DIM + 1  # 65: v columns per head incl ones-column
VP = 80            # padded v stride (DoubleRow needs 16B-aligned pairs)
WARMUP_MMS = 14
DR = mybir.MatmulPerfMode.DoubleRow


def _build_program():
    """Emit the per-core Bass/Tile program (identical on all 8 cores)."""
    nc = bacc.Bacc("TRN2", target_bir_lowering=False, debug=False,
                   num_devices=NCORES)

    xb = nc.dram_tensor("xb", [T, D], X_DT, kind="ExternalInput").ap()
    xt = nc.dram_tensor("xt", [D, T], X_DT, kind="ExternalInput").ap()
    wqkv_t = nc.dram_tensor("wqkv_t", [D, E3], X_DT, kind="ExternalInput").ap()
    wout_t = nc.dram_tensor("wout_t", [HL * HEAD_DIM, D], MM_DT,
                            kind="ExternalInput").ap()
    # per-tile cos/sin in transposed layout, host-replicated: cos 2x per
    # tile ([P, NT*2*128]), sin 4x per tile ([P, NT*4*128], rotate-half
    # sign folded in).
    cos2 = nc.dram_tensor("cos2", [P, NT * 256], TAB_DT, kind="ExternalInput").ap()
    sin4 = nc.dram_tensor("sin4", [P, NT * 512], TAB_DT, kind="ExternalInput").ap()
    triw = nc.dram_tensor("triw", [P, P], FP8, kind="ExternalInput").ap()
    outp = nc.dram_tensor("outp", [T, D], OUT_DT, kind="ExternalOutput").ap()

    with tile.TileContext(nc) as tc:
        _emit(tc, xb, xt, wqkv_t, wout_t, cos2, sin4, triw, outp)

    nc.compile()
    return nc


def _emit(tc, xb, xt, wqkv_t, wout_t, cos2, sin4, triw, outp):
    nc = tc.nc
    from contextlib import ExitStack
    ctx = ExitStack()
    with ctx:
        const = ctx.enter_context(tc.tile_pool(name="const", bufs=1))
        persist = ctx.enter_context(tc.tile_pool(name="persist", bufs=1))
        xin = ctx.enter_context(tc.tile_pool(name="xin", bufs=16))
        hrow = ctx.enter_context(tc.tile_pool(name="hrow", bufs=2))
        stats = ctx.enter_context(tc.tile_pool(name="stats", bufs=14))
        rinvp = ctx.enter_context(tc.tile_pool(name="rinvp", bufs=18))
        qkev = ctx.enter_context(tc.tile_pool(name="qkev", bufs=3))
        rtmp = ctx.enter_context(tc.tile_pool(name="rtmp", bufs=2))
        csin = ctx.enter_context(tc.tile_pool(name="csin", bufs=3))
        ptp = ctx.enter_context(tc.tile_pool(name="ptp", bufs=18))
        nrm = ctx.enter_context(tc.tile_pool(name="nrm", bufs=2))
        orow = ctx.enter_context(tc.tile_pool(name="orow", bufs=3))
        # PSUM budget (8 banks): qkp 1 + vp 1 + tp 1 + sm 2*2 + pv 1
        psp = ctx.enter_context(
            tc.tile_pool(name="psp", bufs=2, space="PSUM"))

        # ---- HAM warmup: dummy matmuls keep the PE activity window busy
        # while the first DMAs land, so real matmuls start at 2.4 GHz.
        wsrc = const.tile([P, 512], MM_DT)
        nc.vector.memset(wsrc[:], 1.0)
        for _ in range(WARMUP_MMS):
            wps = psp.tile([P, 1024], F32, tag="sm")
            nc.tensor.matmul(wps[:, 0:512], wsrc[:, 0:P], wsrc[:],
                             start=True, stop=True)

        # ---- constants / weights resident in SBUF ----
        ident = const.tile([P, P], F32)
        make_identity(nc, ident)
        ident_r = const.tile([P, P], MM_DT)
        nc.scalar.copy(ident_r[:], ident[:])
        tri_sb = const.tile([P, P], FP8)
        nc.sync.dma_start(out=tri_sb[:], in_=triw[:])
        eps_sb = const.tile([P, 1], F32)
        nc.vector.memset(eps_sb[:], float(EPS) * WS * WS)
        zero_sb = const.tile([P, 1], F32)
        nc.vector.memset(zero_sb[:], 0.0)

        # x^T resident: d-block j at cols [T*j].  DMA'd in 512-col chunks;
        # chunk c>0 and the out-proj weights are emitted later, interleaved
        # with the pipeline, so the single sync DMA queue delivers the
        # first tiles' inputs as early as possible.
        xt_sb = persist.tile([P, ND * T], X_DT)
        wq_sb = persist.tile([P, ND * E3], X_DT)    # d-block j at cols [E3*j]
        wo_sb = persist.tile([P, 2 * D], MM_DT)     # d-block j at cols [D*j]

        def dma_xt_chunk(c):
            for j in range(ND):
                nc.sync.dma_start(
                    out=xt_sb[:, T * j + 512 * c:T * j + 512 * (c + 1)],
                    in_=xt[P * j:P * (j + 1), 512 * c:512 * (c + 1)])

        def dma_wo():
            for j in range(2):
                nc.sync.dma_start(out=wo_sb[:, D * j:D * (j + 1)],
                                  in_=wout_t[P * j:P * (j + 1), :])

        for j in range(ND):
            nc.sync.dma_start(out=wq_sb[:, E3 * j:E3 * (j + 1)],
                              in_=wqkv_t[P * j:P * (j + 1), :])
        dma_xt_chunk(0)

        xt_r = xt_sb.rearrange("p (j t) -> p j t", t=T)
        wq_r = wq_sb.rearrange("p (j e) -> p j e", e=E3)

        # Q^T per q-chunk: (128, 2*512); blk j at cols [512j], head h at
        # partitions 64*(h%2) of blk h//2, free = t within the chunk.
        qT_c = [persist.tile([P, 2 * 512], MM_DT, name=f"qT{i}", tag=f"qT{i}")
                for i in range(NQC)]
        # K^T per k-tile: (128, 2*128); blk j at cols [128j].
        kT_t = [persist.tile([P, 2 * P], MM_DT, name=f"kT{i}", tag=f"kT{i}")
                for i in range(NT)]
        # V row-major per k-tile PAIR, fp8: head h at cols [2*VP*h], k-tile
        # ki at sub-block VP*(ki%2), cols 0:64 = v, col 64 = ones.
        v2_t = [persist.tile([P, HL * 2 * VP], FP8, name=f"vT{i}", tag=f"vT{i}")
                for i in range(NT // 2)]
        for pr in range(NT // 2):
            oc = v2_t[pr].rearrange("p (h s c) -> p h s c", s=2, c=VP)[
                :, :, :, HEAD_DIM:VW]
            nc.vector.memset(oc, 1.0)
        # attn-out^T per q-chunk (128, 2*512), laid out like qT_c.
        att_c = [persist.tile([P, 2 * 512], MM_DT, name=f"att{i}", tag=f"att{i}")
                 for i in range(NQC)]

        # ---------------- phase bodies ----------------
        def load_stats(ti):
            """DMA x tile and compute its inverse RMS norm (128,1).

            The 1/rms scale factors out of the QKV contraction, so the raw
            x tile feeds the matmul and the scale is applied per-partition
            during the projection evictions.  The 1/WS weight descale is
            folded in via the sqrt bias/scale."""
            x_t = xin.tile([P, D], X_DT)
            # sync queue: per-tile stats DMA arrives a group ahead of its
            # Square, so the trigger never blocks the ACT FIFO and ACT
            # stops paying ~600ns per trigger instruction
            nc.sync.dma_start(out=x_t[:], in_=xb[P * ti:P * (ti + 1), :])
            sq = hrow.tile([P, D], F32, tag="h")
            ssum = stats.tile([P, 1], F32, tag="ssum")
            nc.scalar.activation(sq[:], x_t[:],
                                 mybir.ActivationFunctionType.Square,
                                 accum_out=ssum[:])
            rstd = stats.tile([P, 1], F32, tag="rstd")
            nc.scalar.activation(rstd[:], ssum[:],
                                 mybir.ActivationFunctionType.Sqrt,
                                 bias=eps_sb[:], scale=WS * WS / D)
            rinv = rinvp.tile([P, 1], F32, tag="rinv")
            nc.vector.reciprocal(rinv[:], rstd[:])
            return rinv

        def phase_a(ti, rinv):
            """QKV projection (fp8 DoubleRow) + transposed-domain RoPE."""
            qc, tloc = ti // 4, ti % 4
            qk_ps = psp.tile([P, 512], F32, tag="qkp", bufs=1)
            v_ps = psp.tile([P, 256], F32, tag="vp", bufs=1)
            for j2 in range(ND // 2):
                lhsT = xt_r[:, 2 * j2:2 * j2 + 2, P * ti:P * (ti + 1)]
                nc.tensor.matmul(qk_ps[:], lhsT,
                                 wq_r[:, 2 * j2:2 * j2 + 2, 0:512],
                                 start=(j2 == 0), stop=(j2 == ND // 2 - 1),
                                 perf_mode=DR)
            # single-op eviction frees the qk psum bank fast (1-buf ring)
            qk_t = qkev.tile([P, 512], MM_DT, tag="qkt")
            nc.vector.tensor_scalar_mul(qk_t[:], qk_ps[:], rinv[:])
            for j2 in range(ND // 2):
                lhsT = xt_r[:, 2 * j2:2 * j2 + 2, P * ti:P * (ti + 1)]
                nc.tensor.matmul(v_ps[:], lhsT,
                                 wq_r[:, 2 * j2:2 * j2 + 2, 512:E3],
                                 start=(j2 == 0), stop=(j2 == ND // 2 - 1),
                                 perf_mode=DR)
            vdst = v2_t[ti // 2].rearrange("p (h s c) -> p h s c",
                                           s=2, c=VP)[:, :, ti % 2, 0:HEAD_DIM]
            vsrc = v_ps[:].rearrange("p (h c) -> p h c", c=HEAD_DIM)
            nc.vector.tensor_scalar_mul(vdst, vsrc, rinv[:])

            # per-tile cos/sin tables in transposed layout; cos replicated
            # 2x (256 cols), sin replicated 4x (512 cols, sign folded).
            # rope tables consumed by DVE, so their triggers can ride the
            # sync queue without risking ACT FIFO-head waits
            ct = csin.tile([P, 256], TAB_DT, tag="ct")
            st = csin.tile([P, 512], TAB_DT, tag="st")
            nc.sync.dma_start(out=ct[:], in_=cos2[:, 256 * ti:256 * (ti + 1)])
            nc.sync.dma_start(out=st[:], in_=sin4[:, 512 * ti:512 * (ti + 1)])

            # rotate-half-shuffled copy, built SBUF->SBUF (off the psum
            # critical path).
            qk_s = qkev.tile([P, 512], MM_DT, tag="qks")
            HH = HEAD_DIM // 2  # 32

            def halves(ap, off):
                return ap.rearrange("p (b i) -> p b i", i=HH)[:, off::2, :]

            nc.vector.tensor_copy(halves(qk_s, 0), halves(qk_t[:], 1))
            nc.vector.tensor_copy(halves(qk_s, 1), halves(qk_t[:], 0))

            # transpose q,k blocks into one (128, 1024) psum tile: A copies
            # (qk_t) at cols [0:512] (q blocks then k blocks), B copies
            # (qk_s) at [512:1024].  RoPE in the transposed domain:
            #   out[p] = A[p]*cos2[p] + B[p]*sin4[p]
            # where sin4 carries the rotate-half sign.
            tp = psp.tile([P, 1024], MM_DT, tag="tp", bufs=1)
            for m in range(4):
                nc.tensor.transpose(tp[:, P * m:P * (m + 1)],
                                    qk_t[:, P * m:P * (m + 1)], ident_r[:])
                nc.tensor.transpose(tp[:, 512 + P * m:512 + P * (m + 1)],
                                    qk_s[:, P * m:P * (m + 1)], ident_r[:])
            tmp = rtmp.tile([P, 512], MM_DT, tag="rt")
            nc.vector.tensor_mul(tmp[:], tp[:, 512:1024], st[:])
            q_ap = qT_c[qc].rearrange("p (b t) -> p b t",
                                      t=512)[:, :, P * tloc:P * (tloc + 1)]
            nc.vector.tensor_mul(q_ap, tp[:, 0:256], ct[:])
            nc.vector.tensor_add(q_ap, q_ap, tmp[:, 0:256])
            k_ap = kT_t[ti][:, 0:256]
            nc.vector.tensor_mul(k_ap, tp[:, 256:512], ct[:])
            nc.vector.tensor_add(k_ap, k_ap, tmp[:, 256:512])

        SC = 0.125  # 1/sqrt(64)

        def st_pass(h, qc):
            """Scores + exp for head h / query chunk qc -> list of pt pair
            tiles.  k-tiles are processed in pairs sharing one 2-bank psum
            tile so a single exp activation covers both; exp writes fp8
            probabilities ready for the DoubleRow PV pass."""
            bp = 64 * (h % 2)
            blk = h // 2
            nki = 4 * qc + 4
            pts = []
            for pr in range(nki // 2):
                ki0, ki1 = 2 * pr, 2 * pr + 1
                z0 = max(0, P * ki0 - 512 * qc)
                z1 = max(0, P * ki1 - 512 * qc)
                sm = psp.tile([P, 1024], F32, tag="sm")
                nc.tensor.matmul(
                    sm[:, z0:512],
                    kT_t[ki0][bp:bp + 64, P * blk:P * (blk + 1)],
                    qT_c[qc][bp:bp + 64, 512 * blk + z0:512 * (blk + 1)],
                    start=True, stop=True)
                # ki1 computes its full query range (even causally-dead
                # columns) so the fused exp below never reads unwritten
                # psum; the dead columns are zeroed after the exp.
                nc.tensor.matmul(
                    sm[:, 512:1024],
                    kT_t[ki1][bp:bp + 64, P * blk:P * (blk + 1)],
                    qT_c[qc][bp:bp + 64, 512 * blk:512 * (blk + 1)],
                    start=True, stop=True)
                pt = ptp.tile([P, 1024], FP8)
                nc.scalar.activation(pt[:, z0:1024], sm[:, z0:1024],
                                     mybir.ActivationFunctionType.Exp,
                                     bias=zero_sb[:], scale=SC)
                if z1 > 0:
                    # exp wrote garbage into ki1's causally-dead columns;
                    # the DoubleRow PV pass streams them, so zero them.
                    nc.vector.memset(pt[:, 512:512 + z1], 0.0)
                if ki1 >= 4 * qc:  # diagonal blocks: apply causal mask
                    if ki0 >= 4 * qc:
                        nc.vector.tensor_mul(pt[:, z0:z0 + P],
                                             pt[:, z0:z0 + P], tri_sb[:])
                    nc.vector.tensor_mul(pt[:, 512 + z1:512 + z1 + P],
                                         pt[:, 512 + z1:512 + z1 + P],
                                         tri_sb[:])
                pts.append((pt, z0, z1))
            return pts

        def pv_pass(pts, g):
            """DoubleRow PV accumulation + softmax normalization for group
            g=(qc,h): each pass contracts a k-tile pair."""
            qc, h = g
            bp = 64 * (h % 2)
            blk = h // 2
            npr = len(pts)
            pv_ps = psp.tile([VW, 512], F32, tag="pv", bufs=1)
            for pr in range(npr):
                pt, z0, z1 = pts[pr]
                lhsT = v2_t[pr].rearrange("p (h s c) -> p h s c",
                                          s=2, c=VP)[:, h, :, 0:VW]
                rhs = pt.rearrange("p (s q) -> p s q", q=512)[:, :, z0:512]
                nc.tensor.matmul(pv_ps[:, z0:512], lhsT, rhs,
                                 start=(pr == 0), stop=(pr == npr - 1),
                                 perf_mode=DR)
            # normalize: rows 0:64 are sum(p*v), row 64 is sum(p).  The
            # denominator row must bounce through SBUF - the iterative
            # reciprocal reading PSUM directly misbehaves on hardware.
            srow = nrm.tile([1, 512], F32, tag="srow")
            nc.vector.tensor_copy(srow[:], pv_ps[64:65, :])
            rrow = nrm.tile([1, 512], F32, tag="rrow")
            nc.vector.reciprocal_approx_fast(rrow[:], srow[:])
            bcast = nrm.tile([64, 512], F32, tag="bcast")
            nc.gpsimd.partition_broadcast(bcast[:], rrow[:])
            nc.vector.tensor_mul(
                att_c[qc][bp:bp + 64, 512 * blk:512 * (blk + 1)],
                pv_ps[0:64, :], bcast[:])

        def phase_c(ti):
            """Partial out-projection for one t-tile."""
            qc, tloc = ti // 4, ti % 4
            o_t = orow.tile([P, D], OUT_DT)
            for ec in range(2):
                if ec == 0:
                    op_ps = psp.tile([P, 512], F32, tag="qkp", bufs=1)
                else:
                    # borrow a score buffer so the two halves double-buffer
                    op_ps = psp.tile([P, 1024], F32, tag="sm")
                for j in range(2):
                    lhs = att_c[qc][:, 512 * j + P * tloc:512 * j + P * (tloc + 1)]
                    nc.tensor.matmul(
                        op_ps[:, 0:512], lhs,
                        wo_sb[:, D * j + 512 * ec:D * j + 512 * (ec + 1)],
                        start=(j == 0), stop=(j == 1))
                nc.vector.tensor_copy(o_t[:, 512 * ec:512 * (ec + 1)],
                                      op_ps[:, 0:512])
            # gpsimd SW-DGE queue: keeps the output stream off the input
            # queues entirely
            nc.gpsimd.dma_start(out=outp[P * ti:P * (ti + 1), :], in_=o_t[:])

        # ---------------- emission ----------------
        # Spread pipeline: at attention group (qc, h) also emit the
        # projection of tile h of chunk qc+1 and the out-projection of
        # tile h of chunk qc-1, so the PE has dense independent work while
        # ACT runs the exps.  PV of group g-1 is emitted after ST of g.
        for ti in range(4):
            phase_a(ti, load_stats(ti))
        dma_xt_chunk(1)
        groups = [(qc, h) for qc in range(NQC) for h in range(HL)]
        prev = None
        for g in groups:
            qc, h = g
            if qc < 2 and h == 0:
                dma_xt_chunk(qc + 2)
            if qc == 0 and h == 1:
                dma_wo()
            nti = 4 * (qc + 1) + h
            if nti < NT:
                phase_a(nti, load_stats(nti))
            pts = st_pass(h, qc)
            if prev is not None:
                pv_pass(*prev)
            if qc > 0:
                phase_c(4 * (qc - 1) + h)
            prev = (pts, g)
        pv_pass(*prev)
        for ti in range(4 * (NQC - 1), NT):
            phase_c(ti)


# ---------------- host-side driver ----------------

_CACHE = {}


def _get_program():
    if "nc" not in _CACHE:
        _CACHE["nc"] = _build_program()
    return _CACHE["nc"]


def _rope_tables():
    half = HEAD_DIM // 2
    inv_freq = (1.0 / (ROPE_BASE ** (np.arange(half, dtype=np.float32) / half))
                ).astype(np.float32)
    pos = np.arange(T, dtype=np.float32)
    freqs = pos[:, None] * inv_freq[None, :]
    emb = np.concatenate([freqs, freqs], axis=-1).astype(np.float32)
    return np.cos(emb).astype(np.float32), np.sin(emb).astype(np.float32)


def make_in_maps(x, norm_w, w_qkv, w_out):
    np_mm = ml_dtypes.bfloat16
    np_x = ml_dtypes.float8_e4m3
    cos, sin = _rope_tables()   # (T, 64) each
    # transposed-domain tables, stacked for two heads per partition block;
    # sin carries the rotate-half sign.  Replicated per tile: cos 2x, sin
    # 4x (per-tile [P,128] blocks repeated along free).
    dhidx = np.arange(P) % HEAD_DIM
    sgn = np.where(dhidx < HEAD_DIM // 2, -1.0, 1.0).astype(np.float32)
    cos2t = np.ascontiguousarray(cos.T[dhidx])              # (128, T)
    sin2t = np.ascontiguousarray(sin.T[dhidx] * sgn[:, None])
    cos2r = np.tile(cos2t.reshape(P, NT, 1, P), (1, 1, 2, 1)).reshape(P, -1)
    sin4r = np.tile(sin2t.reshape(P, NT, 1, P), (1, 1, 4, 1)).reshape(P, -1)
    cos2r = cos2r.astype(ml_dtypes.bfloat16)
    sin4r = sin4r.astype(ml_dtypes.bfloat16)
    tri = (np.arange(P)[None, :] >= np.arange(P)[:, None]).astype(np_x)
    w_fold = (w_qkv * norm_w[None, :]).astype(np.float32) * WS
    in_maps = []
    for c in range(NCORES):
        b, hg = c // 4, c % 4
        sl = slice(256 * hg, 256 * (hg + 1))
        wq = w_fold[0 * D:1 * D][sl]
        wk = w_fold[1 * D:2 * D][sl]
        wv = w_fold[2 * D:3 * D][sl]
        wqkv_c = np.clip(np.ascontiguousarray(
            np.concatenate([wq, wk, wv], axis=0).T), -240, 240).astype(np_x)
        wout_c = np.ascontiguousarray(w_out[:, sl].T).astype(np_mm)
        in_maps.append({
            "xb": np.ascontiguousarray(x[b]).astype(np_x),
            "xt": np.ascontiguousarray(x[b].T).astype(np_x),
            "wqkv_t": wqkv_c,
            "wout_t": wout_c,
            "cos2": cos2r, "sin4": sin4r, "triw": tri,
        })
    return in_maps


def assemble(x, results):
    out = np.empty((B, T, D), dtype=np.float32)
    for b in range(B):
        acc = x[b].astype(np.float32).copy()
        for hg in range(4):
            acc += results[4 * b + hg]["outp"].astype(np.float32)
        out[b] = acc
    return out


def kernel(x, norm_w, w_qkv, w_out, trace=False):
    x = np.asarray(x, dtype=np.float32)
    norm_w = np.asarray(norm_w, dtype=np.float32)
    w_qkv = np.asarray(w_qkv, dtype=np.float32)
    w_out = np.asarray(w_out, dtype=np.float32)
    nc = _get_program()
    in_maps = make_in_maps(x, norm_w, w_qkv, w_out)
    res = run_bass_kernel_spmd(nc, in_maps, core_ids=list(range(NCORES)),
                               trace=trace)
    _CACHE["last_results"] = res
    return assemble(x, res.results)
